# revision 3
# baseline (speedup 1.0000x reference)
"""Per-domain batch normalization (BaseDomainBatchNorm) on 8 Trainium2 NeuronCores.

Math (reference):
    cnt[j]   = #{n : d[n] == j}            (clamped to >= 1)
    mean[j]  = sum_{d[n]==j} X[n] / cnt[j]
    var[j]   = sum_{d[n]==j} X[n]^2 / cnt[j] - mean[j]^2
    inv[j]   = rsqrt(var[j] + 1e-5)
    Y[n]     = (X[n] - mean[d[n]]) * inv[d[n]] * gamma[d[n]] + beta[d[n]]
             = X[n] * A[d[n]] + B[d[n]],  A = inv*gamma, B = beta - mean*A

Sharding: rows (samples) split 8192 per core; per-domain partial stats
(sum / sumsq / count) are AllReduce'd (fp16 payload) across the 8 cores;
each core then normalizes its own rows.  gamma/beta replicated.

On-core schedule (fp16 data paths, fp32 accumulation):
  phase 1 (DMA-bound): X streams in per 256-row super-chunk; DVE casts
    f32->fp16 (2x two-port mode), squares alternate DVE (fp16 2x) and
    ScalarE (Square activation); stats accumulate in PSUM via one-hot
    fp16 matmuls.  GpSimd builds the transposed one-hot for phase 2.
  all-reduce: [16, 1025] fp16 payload over shared DRAM; PE kept warm by
    a chained junk-matmul ladder; ScalarE preloads Log/Exp act tables.
  finalize: inv = exp(-0.5*log(var+eps)) on ScalarE; A/B tables in fp16.
  phase 2: per chunk one K=32 gather matmul pair (A,B) -> PSUM f32;
    3/4 of supers: ScalarE copies PSUM->fp16, DVE does fp16 2x mul/add;
    1/4 of supers: DVE consumes PSUM f32 directly (1x).  Y leaves as
    fp16 (host upcasts), halving write traffic.
"""

import numpy as np

N = 65536
C = 512
D = 16
NCORES = 8
SHARD = N // NCORES          # 8192 rows per core
P = 128                      # partitions
CHUNKS = SHARD // P          # 64 chunks of 128 rows
SUPERS = CHUNKS // 2         # 32 super-chunks of 256 rows
EPS = 1e-5
KT = 32                      # gather stationary K (table rows 0:16 + zero pad)

_CACHE = {}


def _build_program():
    import concourse.bacc as bacc
    import concourse.bass as bass
    import concourse.tile as tile
    from concourse import mybir

    f32 = mybir.dt.float32
    f16 = mybir.dt.float16
    i32 = mybir.dt.int32
    Alu = mybir.AluOpType
    Act = mybir.ActivationFunctionType

    nc = bacc.Bacc("TRN2", target_bir_lowering=False, debug=False,
                   num_devices=NCORES)

    X_d = nc.dram_tensor("X", [SHARD, C], f32, kind="ExternalInput")
    d_d = nc.dram_tensor("d", [SHARD], i32, kind="ExternalInput")
    g_d = nc.dram_tensor("gamma", [D, C], f32, kind="ExternalInput")
    b_d = nc.dram_tensor("beta", [D, C], f32, kind="ExternalInput")
    Y_d = nc.dram_tensor("Y", [SHARD, C], f16, kind="ExternalOutput")

    cc_in = nc.dram_tensor("cc_in", [D, 2 * C + 1], f16)
    cc_out = nc.dram_tensor("cc_out", [D, 2 * C + 1], f16, addr_space="Shared")

    # partition p owns rows [p*64, (p+1)*64): per-partition contiguous DMA
    Xv = X_d.ap().rearrange("(p n) c -> p n c", p=P)   # [128, 64, 512]
    Yv = Y_d.ap().rearrange("(p n) c -> p n c", p=P)

    DB = 1024  # d-broadcast strip width

    with tile.TileContext(nc) as tc:
        with (
            tc.tile_pool(name="const", bufs=1) as cpool,
            tc.tile_pool(name="x", bufs=4) as xpool,
            tc.tile_pool(name="xb", bufs=SUPERS) as xbpool,
            tc.tile_pool(name="sq", bufs=4) as sqpool,
            tc.tile_pool(name="oh", bufs=1) as ohpool,
            tc.tile_pool(name="small", bufs=1) as spool,
            tc.tile_pool(name="scr", bufs=2) as scrpool,
            tc.tile_pool(name="dbc", bufs=2) as dbcpool,
            tc.tile_pool(name="pt", bufs=4) as ptpool,
            tc.tile_pool(name="y", bufs=4) as ypool,
        ):
            # ---- constants ----
            # iota_rep[p, i, j] = j  (for the chunk-layout one-hot)
            iota_rep = cpool.tile([P, CHUNKS, D], f16)
            nc.gpsimd.iota(iota_rep[:], pattern=[[0, CHUNKS], [1, D]], base=0,
                           channel_multiplier=0,
                           allow_small_or_imprecise_dtypes=True)
            # iota_col32[p, 0] = p % 16 as f32 (for the transposed one-hot)
            iota_i = cpool.tile([KT, 1], i32)
            nc.gpsimd.iota(iota_i[:], pattern=[[0, 1]], base=0,
                           channel_multiplier=1)
            nc.vector.tensor_scalar(iota_i[:], iota_i[:], D - 1, None,
                                    Alu.bitwise_and)
            iota_col32 = cpool.tile([KT, 1], f32)
            nc.vector.tensor_copy(iota_col32[:], iota_i[:])
            ones_col = cpool.tile([P, 1], f16)
            nc.vector.memset(ones_col[:], 1.0)
            epsb = cpool.tile([D, 1], f32)
            nc.vector.memset(epsb[:], EPS)

            # ---- d in chunk layout ([p, n]) and one-hot [128, 64, 16] ----
            d_pn = cpool.tile([P, CHUNKS], i32)
            nc.sync.dma_start(d_pn[:], d_d.ap().rearrange("(p n) -> p n", p=P))
            d_f = cpool.tile([P, CHUNKS], f16)
            nc.vector.tensor_copy(d_f[:], d_pn[:])
            onehot = ohpool.tile([P, CHUNKS, D], f16)
            nc.vector.tensor_tensor(
                onehot[:], iota_rep[:],
                d_f[:].unsqueeze(-1).broadcast_to([P, CHUNKS, D]),
                Alu.is_equal)

            # ---- transposed one-hot, rows 0:16 real / 16:32 dup (their
            # table rows in A2/B2 are zero); gather matmuls use K=32 ----
            onehotT = ohpool.tile([KT, SHARD], f16)
            for h in range(SHARD // DB):
                d_bc = dbcpool.tile([KT, DB], i32)
                src = d_d.ap()[h * DB:(h + 1) * DB]
                src = src.rearrange("(a n) -> a n", a=1).partition_broadcast(KT)
                nc.gpsimd.dma_start(d_bc[:], src)
                nc.gpsimd.tensor_scalar(onehotT[:, h * DB:(h + 1) * DB],
                                        d_bc[:], iota_col32[:], None,
                                        Alu.is_equal)

            # fp16 A/B tables, rows 16:32 zero
            A2 = spool.tile([KT, C], f16, tag="A2")
            B2 = spool.tile([KT, C], f16, tag="B2")
            nc.vector.memset(A2[:], 0.0)
            nc.vector.memset(B2[:], 0.0)

            # gamma/beta in early (needed post-AR)
            gam = spool.tile([D, C], f32, tag="gam")
            nc.scalar.dma_start(gam[:], g_d[:])
            bet = spool.tile([D, C], f32, tag="bet")
            nc.scalar.dma_start(bet[:], b_d[:])

            # ---- phase 1: per-core partial stats ----
            stats = spool.tile([D, 2 * C + 1], f16, tag="stats")
            xbs = []
            with tc.tile_pool(name="ps1", bufs=1, space="PSUM") as ps1:
                psum_s = ps1.tile([D, C], f32)
                psum_q = ps1.tile([D, C], f32)
                psum_c = ps1.tile([D, 1], f32)
                for s in range(SUPERS):
                    xt = xpool.tile([P, 2 * C], f32)
                    nc.sync.dma_start(
                        xt[:].rearrange("p (n c) -> p n c", c=C),
                        Xv[:, 2 * s:2 * s + 2, :])
                    xb = xbpool.tile([P, 2 * C], f16)
                    xbs.append(xb)
                    nc.vector.tensor_copy(xb[:], xt[:])
                    xq = sqpool.tile([P, 2 * C], f16, tag="xq")
                    if s % 2 == 0:
                        nc.scalar.activation(xq[:], xt[:], Act.Square)
                    else:
                        nc.vector.tensor_mul(xq[:], xb[:], xb[:])
                    for k in range(2):
                        i = 2 * s + k
                        oh = onehot[:, i, :]
                        st, sp = (i == 0), (i == CHUNKS - 1)
                        csl = slice(k * C, (k + 1) * C)
                        nc.tensor.matmul(psum_s[:], oh, xb[:, csl],
                                         start=st, stop=sp)
                        nc.tensor.matmul(psum_q[:], oh, xq[:, csl],
                                         start=st, stop=sp)

                # counts: reduce one-hot over chunks, then one matmul
                rowcnt = spool.tile([P, D], f32, tag="rowcnt")
                nc.vector.tensor_reduce(
                    rowcnt[:], onehot[:].rearrange("p n d -> p d n"),
                    mybir.AxisListType.X, Alu.add)
                rowcnt16 = spool.tile([P, D], f16, tag="rowcnt16")
                nc.vector.tensor_copy(rowcnt16[:], rowcnt[:])
                nc.tensor.matmul(psum_c[:], rowcnt16[:], ones_col[:],
                                 start=True, stop=True)

                # ---- pack fp16 stats out of PSUM ----
                nc.vector.tensor_copy(stats[:, 0:C], psum_s[:])
                nc.vector.tensor_copy(stats[:, C:2 * C], psum_q[:])
                nc.vector.tensor_copy(stats[:, 2 * C:2 * C + 1], psum_c[:])

                # ---- all-reduce partial stats across the 8 cores ----
                nc.sync.dma_start(cc_in[:], stats[:])
                nc.gpsimd.collective_compute(
                    "AllReduce", Alu.add,
                    replica_groups=[list(range(NCORES))],
                    ins=[cc_in[:]], outs=[cc_out[:]])

                # keep the PE HAM clock-gate warm across the all-reduce
                # stall: junk matmuls chained through ScalarE copies so
                # they spread over the stall instead of firing at once
                warm = ps1.tile([D, C], f32)
                wsb = spool.tile([D, C], f16, tag="wsb")
                nc.vector.memset(wsb[:], 1.0)
                for w in range(24):
                    nc.tensor.matmul(warm[:], onehot[:, w, :],
                                     xbs[0][:, 0:C],
                                     start=True, stop=True,
                                     skip_group_check=True)
                    nc.scalar.activation(wsb[:], warm[:], Act.Copy)

                # ScalarE: preload Log/Exp tables during the AR stall
                tdummy = spool.tile([1, 1], f32, tag="tdummy")
                nc.scalar.activation(tdummy[:], epsb[0:1, :], Act.Ln,
                                     bias=epsb[0:1, :])
                nc.scalar.activation(tdummy[:], tdummy[:], Act.Exp)

            red = spool.tile([D, 2 * C + 1], f16, tag="red")
            nc.sync.dma_start(red[:], cc_out[:])

            # ---- finalize: A = inv*gamma, B = beta - mean*A ----
            redf = spool.tile([D, 2 * C + 1], f32, tag="redf")
            nc.vector.tensor_copy(redf[:], red[:])
            cntc = spool.tile([D, 1], f32, tag="cntc")
            nc.vector.tensor_scalar_max(cntc[:], redf[:, 2 * C:2 * C + 1], 1.0)
            rinv = spool.tile([D, 1], f32, tag="rinv")
            nc.vector.reciprocal(rinv[:], cntc[:])
            mean = spool.tile([D, C], f32, tag="mean")
            nc.vector.tensor_scalar_mul(mean[:], redf[:, 0:C], rinv[:])
            var = spool.tile([D, C], f32, tag="var")
            nc.vector.tensor_scalar_mul(var[:], redf[:, C:2 * C], rinv[:])
            negm2 = scrpool.tile([D, C], f32, tag="scr")
            nc.vector.scalar_tensor_tensor(negm2[:], mean[:], -1.0, mean[:],
                                           Alu.mult, Alu.mult)
            nc.vector.tensor_add(var[:], var[:], negm2[:])
            # inv = exp(-0.5 * log(var + eps))  (tables preloaded above)
            lv = scrpool.tile([D, C], f32, tag="scr")
            nc.scalar.activation(lv[:], var[:], Act.Ln, bias=epsb[:])
            inv = spool.tile([D, C], f32, tag="inv")
            nc.scalar.activation(inv[:], lv[:], Act.Exp, scale=-0.5)

            a_t = spool.tile([D, C], f32, tag="a_t")
            nc.vector.tensor_mul(a_t[:], inv[:], gam[:])
            nc.vector.tensor_copy(A2[0:D, :], a_t[:])
            b_t = spool.tile([D, C], f32, tag="b_t")
            nc.vector.scalar_tensor_tensor(b_t[:], mean[:], -1.0, a_t[:],
                                           Alu.mult, Alu.mult)   # -mean*A
            nc.vector.tensor_add(b_t[:], bet[:], b_t[:])
            nc.vector.tensor_copy(B2[0:D, :], b_t[:])

            # ---- phase 2: gather A/B per row and normalize ----
            ohT = onehotT[:].rearrange("k (p i) -> k i p", i=CHUNKS)
            with tc.tile_pool(name="ps2", bufs=2, space="PSUM") as ps2:
                for s in range(SUPERS):
                    pa = ps2.tile([P, 2 * C], f32)
                    pb = ps2.tile([P, 2 * C], f32)
                    for k in range(2):
                        i = 2 * s + k
                        lt = ohT[:, i, :]
                        csl = slice(k * C, (k + 1) * C)
                        nc.tensor.matmul(pa[:, csl], lt, A2[:],
                                         start=True, stop=True)
                        nc.tensor.matmul(pb[:, csl], lt, B2[:],
                                         start=True, stop=True)
                    yt = ypool.tile([P, 2 * C], f16)
                    if s % 4 == 1:
                        # DVE consumes PSUM f32 directly (1x mode)
                        nc.vector.tensor_mul(yt[:], xbs[s][:], pa[:])
                        nc.vector.tensor_add(yt[:], yt[:], pb[:])
                    else:
                        # ScalarE drains PSUM to fp16; DVE runs 2x fp16
                        pa16 = ptpool.tile([P, 2 * C], f16)
                        nc.scalar.activation(pa16[:], pa[:], Act.Copy)
                        pb16 = ptpool.tile([P, 2 * C], f16)
                        nc.scalar.activation(pb16[:], pb[:], Act.Copy)
                        nc.vector.tensor_mul(yt[:], xbs[s][:], pa16[:])
                        nc.vector.tensor_add(yt[:], yt[:], pb16[:])
                    nc.sync.dma_start(
                        Yv[:, 2 * s:2 * s + 2, :],
                        yt[:].rearrange("p (n c) -> p n c", c=C))

    nc.compile()
    return nc


def _get_program():
    if "nc" not in _CACHE:
        _CACHE["nc"] = _build_program()
    return _CACHE["nc"]


def kernel(X, d, parameter_t, fm_mean, gamma, beta):
    from concourse.bass_utils import run_bass_kernel_spmd

    X = np.ascontiguousarray(np.asarray(X), dtype=np.float32)
    d = np.ascontiguousarray(np.asarray(d), dtype=np.int32)
    gamma = np.ascontiguousarray(np.asarray(gamma), dtype=np.float32)
    beta = np.ascontiguousarray(np.asarray(beta), dtype=np.float32)

    nc = _get_program()
    in_maps = [
        {
            "X": X[c * SHARD:(c + 1) * SHARD],
            "d": d[c * SHARD:(c + 1) * SHARD],
            "gamma": gamma,
            "beta": beta,
        }
        for c in range(NCORES)
    ]
    res = run_bass_kernel_spmd(nc, in_maps, core_ids=list(range(NCORES)))
    out = np.concatenate([res.results[c]["Y"] for c in range(NCORES)], axis=0)
    return out.astype(np.float32, copy=False)


# revision 5
# speedup vs baseline: 1.4579x; 1.4579x over previous
"""Per-domain batch normalization (BaseDomainBatchNorm) on 8 Trainium2 NeuronCores.

Math (reference):
    cnt[j]   = #{n : d[n] == j}            (clamped to >= 1)
    mean[j]  = sum_{d[n]==j} X[n] / cnt[j]
    var[j]   = sum_{d[n]==j} X[n]^2 / cnt[j] - mean[j]^2
    inv[j]   = rsqrt(var[j] + 1e-5)
    Y[n]     = (X[n] - mean[d[n]]) * inv[d[n]] * gamma[d[n]] + beta[d[n]]
             = X[n] * A[d[n]] + B[d[n]],  A = inv*gamma, B = beta - mean*A

Sharding: rows (samples) split 8192 per core; per-domain partial stats
(sum / sumsq / count) are AllReduce'd (fp16 payload) across the 8 cores;
each core then normalizes its own rows.  gamma/beta replicated.

On-core schedule (fp16 data paths, fp32 accumulation):
  phase 1 (DMA-bound): X streams in per 256-row super-chunk; DVE casts
    f32->fp16 (2x two-port mode), squares alternate DVE (fp16 2x) and
    ScalarE (Square activation); stats accumulate in PSUM via one-hot
    fp16 matmuls.  GpSimd builds the transposed one-hot for phase 2.
  all-reduce: [16, 1025] fp16 payload over shared DRAM; PE kept warm by
    a chained junk-matmul ladder; ScalarE preloads Log/Exp act tables.
  finalize: inv = exp(-0.5*log(var+eps)) on ScalarE; A/B tables in fp16.
  phase 2: per chunk one K=32 gather matmul pair (A,B) -> PSUM f32;
    3/4 of supers: ScalarE copies PSUM->fp16, DVE does fp16 2x mul/add;
    1/4 of supers: DVE consumes PSUM f32 directly (1x).  Y leaves as
    fp16 (host upcasts), halving write traffic.
"""

import numpy as np

N = 65536
C = 512
D = 16
NCORES = 8
SHARD = N // NCORES          # 8192 rows per core
P = 128                      # partitions
CHUNKS = SHARD // P          # 64 chunks of 128 rows
SUPERS = CHUNKS // 2         # 32 super-chunks of 256 rows
EPS = 1e-5
KT = 32                      # gather stationary K (table rows 0:16 + zero pad)

_CACHE = {}


def _build_program():
    import concourse.bacc as bacc
    import concourse.bass as bass
    import concourse.tile as tile
    from concourse import mybir

    f32 = mybir.dt.float32
    f16 = mybir.dt.float16
    i32 = mybir.dt.int32
    Alu = mybir.AluOpType
    Act = mybir.ActivationFunctionType

    nc = bacc.Bacc("TRN2", target_bir_lowering=False, debug=False,
                   num_devices=NCORES)

    X_d = nc.dram_tensor("X", [SHARD, C], f32, kind="ExternalInput")
    d_d = nc.dram_tensor("d", [SHARD], i32, kind="ExternalInput")
    g_d = nc.dram_tensor("gamma", [D, C], f32, kind="ExternalInput")
    b_d = nc.dram_tensor("beta", [D, C], f32, kind="ExternalInput")
    Y_d = nc.dram_tensor("Y", [SHARD, C], f16, kind="ExternalOutput")

    cc_in = nc.dram_tensor("cc_in", [D, 2 * C + 1], f16)
    cc_out = nc.dram_tensor("cc_out", [D, 2 * C + 1], f16, addr_space="Shared")

    # partition p owns rows [p*64, (p+1)*64): per-partition contiguous DMA
    Xv = X_d.ap().rearrange("(p n) c -> p n c", p=P)   # [128, 64, 512]
    Yv = Y_d.ap().rearrange("(p n) c -> p n c", p=P)

    DB = 1024  # d-broadcast strip width

    with tile.TileContext(nc) as tc:
        with (
            tc.tile_pool(name="const", bufs=1) as cpool,
            tc.tile_pool(name="x", bufs=4) as xpool,
            tc.tile_pool(name="xb", bufs=SUPERS) as xbpool,
            tc.tile_pool(name="sq", bufs=4) as sqpool,
            tc.tile_pool(name="oh", bufs=1) as ohpool,
            tc.tile_pool(name="small", bufs=1) as spool,
            tc.tile_pool(name="scr", bufs=2) as scrpool,
            tc.tile_pool(name="dbc", bufs=2) as dbcpool,
            tc.tile_pool(name="pt", bufs=4) as ptpool,
            tc.tile_pool(name="y", bufs=4) as ypool,
        ):
            # ---- constants ----
            # iota_rep[p, i, j] = j  (for the chunk-layout one-hot)
            iota_rep = cpool.tile([P, CHUNKS, D], f16)
            nc.gpsimd.iota(iota_rep[:], pattern=[[0, CHUNKS], [1, D]], base=0,
                           channel_multiplier=0,
                           allow_small_or_imprecise_dtypes=True)
            # iota_col32[p, 0] = p % 16 as f32 (for the transposed one-hot)
            iota_i = cpool.tile([KT, 1], i32)
            nc.gpsimd.iota(iota_i[:], pattern=[[0, 1]], base=0,
                           channel_multiplier=1)
            nc.vector.tensor_scalar(iota_i[:], iota_i[:], D - 1, None,
                                    Alu.bitwise_and)
            iota_col32 = cpool.tile([KT, 1], f32)
            nc.vector.tensor_copy(iota_col32[:], iota_i[:])
            ones_col = cpool.tile([P, 1], f16)
            nc.vector.memset(ones_col[:], 1.0)
            epsb = cpool.tile([D, 1], f32)
            nc.vector.memset(epsb[:], EPS)

            # ---- d in chunk layout ([p, n]) and one-hot [128, 64, 16] ----
            d_pn = cpool.tile([P, CHUNKS], i32)
            nc.sync.dma_start(d_pn[:], d_d.ap().rearrange("(p n) -> p n", p=P))
            d_f = cpool.tile([P, CHUNKS], f16)
            nc.vector.tensor_copy(d_f[:], d_pn[:])
            onehot = ohpool.tile([P, CHUNKS, D], f16)
            nc.vector.tensor_tensor(
                onehot[:], iota_rep[:],
                d_f[:].unsqueeze(-1).broadcast_to([P, CHUNKS, D]),
                Alu.is_equal)

            # ---- transposed one-hot, rows 0:16 real / 16:32 dup (their
            # table rows in A2/B2 are zero); gather matmuls use K=32.
            # Strips are built lazily inside the phase-1 loop (DVE slack),
            # NOT on gpsimd: its elementwise path is ~20x slower and it
            # shares the DVE SBUF port. ----
            onehotT = ohpool.tile([KT, SHARD], f16)

            def emit_strip(h):
                d_bc = dbcpool.tile([KT, DB], i32)
                src = d_d.ap()[h * DB:(h + 1) * DB]
                src = src.rearrange("(a n) -> a n", a=1).partition_broadcast(KT)
                nc.gpsimd.dma_start(d_bc[:], src)
                nc.vector.tensor_scalar(onehotT[:, h * DB:(h + 1) * DB],
                                        d_bc[:], iota_col32[:], None,
                                        Alu.is_equal)

            # fp16 A/B tables, rows 16:32 zero
            A2 = spool.tile([KT, C], f16, tag="A2")
            B2 = spool.tile([KT, C], f16, tag="B2")
            nc.vector.memset(A2[:], 0.0)
            nc.vector.memset(B2[:], 0.0)

            # gamma/beta in early (needed post-AR)
            gam = spool.tile([D, C], f32, tag="gam")
            nc.scalar.dma_start(gam[:], g_d[:])
            bet = spool.tile([D, C], f32, tag="bet")
            nc.scalar.dma_start(bet[:], b_d[:])

            # ---- phase 1: per-core partial stats ----
            stats = spool.tile([D, 2 * C + 1], f16, tag="stats")
            xbs = []
            with tc.tile_pool(name="ps1", bufs=1, space="PSUM") as ps1:
                psum_s = ps1.tile([D, C], f32)
                psum_q = ps1.tile([D, C], f32)
                psum_c = ps1.tile([D, 1], f32)
                strip_at = {2 + 3 * h: h for h in range(SHARD // DB)}
                for s in range(SUPERS):
                    xt = xpool.tile([P, 2 * C], f32)
                    nc.sync.dma_start(
                        xt[:].rearrange("p (n c) -> p n c", c=C),
                        Xv[:, 2 * s:2 * s + 2, :])
                    xb = xbpool.tile([P, 2 * C], f16)
                    xbs.append(xb)
                    nc.vector.tensor_copy(xb[:], xt[:])
                    xq = sqpool.tile([P, 2 * C], f16, tag="xq")
                    nc.scalar.activation(xq[:], xt[:], Act.Square)
                    if s in strip_at:
                        emit_strip(strip_at[s])
                    for k in range(2):
                        i = 2 * s + k
                        oh = onehot[:, i, :]
                        st, sp = (i == 0), (i == CHUNKS - 1)
                        csl = slice(k * C, (k + 1) * C)
                        nc.tensor.matmul(psum_s[:], oh, xb[:, csl],
                                         start=st, stop=sp)
                        nc.tensor.matmul(psum_q[:], oh, xq[:, csl],
                                         start=st, stop=sp)

                # counts: reduce one-hot over chunks, then one matmul
                rowcnt = spool.tile([P, D], f32, tag="rowcnt")
                nc.vector.tensor_reduce(
                    rowcnt[:], onehot[:].rearrange("p n d -> p d n"),
                    mybir.AxisListType.X, Alu.add)
                rowcnt16 = spool.tile([P, D], f16, tag="rowcnt16")
                nc.vector.tensor_copy(rowcnt16[:], rowcnt[:])
                nc.tensor.matmul(psum_c[:], rowcnt16[:], ones_col[:],
                                 start=True, stop=True)

                # ---- pack fp16 stats out of PSUM ----
                nc.vector.tensor_copy(stats[:, 0:C], psum_s[:])
                nc.vector.tensor_copy(stats[:, C:2 * C], psum_q[:])
                nc.vector.tensor_copy(stats[:, 2 * C:2 * C + 1], psum_c[:])

                # ---- all-reduce partial stats across the 8 cores ----
                nc.sync.dma_start(cc_in[:], stats[:])
                nc.gpsimd.collective_compute(
                    "AllReduce", Alu.add,
                    replica_groups=[list(range(NCORES))],
                    ins=[cc_in[:]], outs=[cc_out[:]])

                # keep the PE HAM clock-gate warm across the all-reduce
                # stall: junk matmuls chained through ScalarE copies so
                # they spread over the stall instead of firing at once
                warm = ps1.tile([D, C], f32)
                wsb = spool.tile([D, C], f16, tag="wsb")
                nc.vector.memset(wsb[:], 1.0)
                for w in range(24):
                    nc.tensor.matmul(warm[:], onehot[:, w, :],
                                     xbs[0][:, 0:C],
                                     start=True, stop=True,
                                     skip_group_check=True)
                    nc.scalar.activation(wsb[:], warm[:], Act.Copy)

                # ScalarE: preload Log/Exp tables during the AR stall
                tdummy = spool.tile([1, 1], f32, tag="tdummy")
                nc.scalar.activation(tdummy[:], epsb[0:1, :], Act.Ln,
                                     bias=epsb[0:1, :])
                nc.scalar.activation(tdummy[:], tdummy[:], Act.Exp)

            red = spool.tile([D, 2 * C + 1], f16, tag="red")
            nc.sync.dma_start(red[:], cc_out[:])

            # ---- finalize: A = inv*gamma, B = beta - mean*A ----
            redf = spool.tile([D, 2 * C + 1], f32, tag="redf")
            nc.vector.tensor_copy(redf[:], red[:])
            cntc = spool.tile([D, 1], f32, tag="cntc")
            nc.vector.tensor_scalar_max(cntc[:], redf[:, 2 * C:2 * C + 1], 1.0)
            rinv = spool.tile([D, 1], f32, tag="rinv")
            nc.vector.reciprocal(rinv[:], cntc[:])
            mean = spool.tile([D, C], f32, tag="mean")
            nc.vector.tensor_scalar_mul(mean[:], redf[:, 0:C], rinv[:])
            var = spool.tile([D, C], f32, tag="var")
            nc.vector.tensor_scalar_mul(var[:], redf[:, C:2 * C], rinv[:])
            negm2 = scrpool.tile([D, C], f32, tag="scr")
            nc.vector.scalar_tensor_tensor(negm2[:], mean[:], -1.0, mean[:],
                                           Alu.mult, Alu.mult)
            nc.vector.tensor_add(var[:], var[:], negm2[:])
            # inv = exp(-0.5 * log(var + eps))  (tables preloaded above)
            lv = scrpool.tile([D, C], f32, tag="scr")
            nc.scalar.activation(lv[:], var[:], Act.Ln, bias=epsb[:])
            inv = spool.tile([D, C], f32, tag="inv")
            nc.scalar.activation(inv[:], lv[:], Act.Exp, scale=-0.5)

            a_t = spool.tile([D, C], f32, tag="a_t")
            nc.vector.tensor_mul(a_t[:], inv[:], gam[:])
            nc.vector.tensor_copy(A2[0:D, :], a_t[:])
            b_t = spool.tile([D, C], f32, tag="b_t")
            nc.vector.scalar_tensor_tensor(b_t[:], mean[:], -1.0, a_t[:],
                                           Alu.mult, Alu.mult)   # -mean*A
            nc.vector.tensor_add(b_t[:], bet[:], b_t[:])
            nc.vector.tensor_copy(B2[0:D, :], b_t[:])

            # ---- phase 2: gather A/B per row and normalize ----
            ohT = onehotT[:].rearrange("k (p i) -> k i p", i=CHUNKS)
            with tc.tile_pool(name="ps2", bufs=2, space="PSUM") as ps2:
                for s in range(SUPERS):
                    pa = ps2.tile([P, 2 * C], f32)
                    pb = ps2.tile([P, 2 * C], f32)
                    for k in range(2):
                        i = 2 * s + k
                        lt = ohT[:, i, :]
                        csl = slice(k * C, (k + 1) * C)
                        nc.tensor.matmul(pa[:, csl], lt, A2[:],
                                         start=True, stop=True)
                        nc.tensor.matmul(pb[:, csl], lt, B2[:],
                                         start=True, stop=True)
                    yt = ypool.tile([P, 2 * C], f16)
                    if s % 4 == 1:
                        # DVE consumes PSUM f32 directly (1x mode)
                        nc.vector.tensor_mul(yt[:], xbs[s][:], pa[:])
                        nc.vector.tensor_add(yt[:], yt[:], pb[:])
                    else:
                        # ScalarE drains PSUM to fp16; DVE runs 2x fp16
                        pa16 = ptpool.tile([P, 2 * C], f16)
                        nc.scalar.activation(pa16[:], pa[:], Act.Copy)
                        pb16 = ptpool.tile([P, 2 * C], f16)
                        nc.scalar.activation(pb16[:], pb[:], Act.Copy)
                        nc.vector.tensor_mul(yt[:], xbs[s][:], pa16[:])
                        nc.vector.tensor_add(yt[:], yt[:], pb16[:])
                    nc.sync.dma_start(
                        Yv[:, 2 * s:2 * s + 2, :],
                        yt[:].rearrange("p (n c) -> p n c", c=C))

    nc.compile()
    return nc


def _get_program():
    if "nc" not in _CACHE:
        _CACHE["nc"] = _build_program()
    return _CACHE["nc"]


def kernel(X, d, parameter_t, fm_mean, gamma, beta):
    from concourse.bass_utils import run_bass_kernel_spmd

    X = np.ascontiguousarray(np.asarray(X), dtype=np.float32)
    d = np.ascontiguousarray(np.asarray(d), dtype=np.int32)
    gamma = np.ascontiguousarray(np.asarray(gamma), dtype=np.float32)
    beta = np.ascontiguousarray(np.asarray(beta), dtype=np.float32)

    nc = _get_program()
    in_maps = [
        {
            "X": X[c * SHARD:(c + 1) * SHARD],
            "d": d[c * SHARD:(c + 1) * SHARD],
            "gamma": gamma,
            "beta": beta,
        }
        for c in range(NCORES)
    ]
    res = run_bass_kernel_spmd(nc, in_maps, core_ids=list(range(NCORES)))
    out = np.concatenate([res.results[c]["Y"] for c in range(NCORES)], axis=0)
    return out.astype(np.float32, copy=False)


# revision 7
# speedup vs baseline: 1.5207x; 1.0430x over previous
"""Per-domain batch normalization (BaseDomainBatchNorm) on 8 Trainium2 NeuronCores.

Math (reference):
    cnt[j]   = #{n : d[n] == j}            (clamped to >= 1)
    mean[j]  = sum_{d[n]==j} X[n] / cnt[j]
    var[j]   = sum_{d[n]==j} X[n]^2 / cnt[j] - mean[j]^2
    inv[j]   = rsqrt(var[j] + 1e-5)
    Y[n]     = (X[n] - mean[d[n]]) * inv[d[n]] * gamma[d[n]] + beta[d[n]]
             = X[n] * A[d[n]] + B[d[n]],  A = inv*gamma, B = beta - mean*A

Sharding: rows (samples) split 8192 per core; per-domain partial stats
(sum / sumsq / count) are AllReduce'd (fp16 payload) across the 8 cores;
each core then normalizes its own rows.  gamma/beta replicated.

On-core schedule (fp16 data paths, fp32 accumulation):
  phase 1 (DMA-bound): X streams in per 256-row super-chunk; DVE casts
    f32->fp16 (2x two-port mode), squares alternate DVE (fp16 2x) and
    ScalarE (Square activation); stats accumulate in PSUM via one-hot
    fp16 matmuls.  GpSimd builds the transposed one-hot for phase 2.
  all-reduce: [16, 1025] fp16 payload over shared DRAM; PE kept warm by
    a chained junk-matmul ladder; ScalarE preloads Log/Exp act tables.
  finalize: inv = exp(-0.5*log(var+eps)) on ScalarE; A/B tables in fp16.
  phase 2: per chunk one K=32 gather matmul pair (A,B) -> PSUM f32;
    3/4 of supers: ScalarE copies PSUM->fp16, DVE does fp16 2x mul/add;
    1/4 of supers: DVE consumes PSUM f32 directly (1x).  Y leaves as
    fp16 (host upcasts), halving write traffic.
"""

import numpy as np

N = 65536
C = 512
D = 16
NCORES = 8
SHARD = N // NCORES          # 8192 rows per core
P = 128                      # partitions
CHUNKS = SHARD // P          # 64 chunks of 128 rows
SUPERS = CHUNKS // 2         # 32 super-chunks of 256 rows
EPS = 1e-5
KT = 32                      # gather stationary K (table rows 0:16 + zero pad)

_CACHE = {}


def _build_program():
    import concourse.bacc as bacc
    import concourse.bass as bass
    import concourse.tile as tile
    from concourse import mybir

    f32 = mybir.dt.float32
    f16 = mybir.dt.float16
    i32 = mybir.dt.int32
    Alu = mybir.AluOpType
    Act = mybir.ActivationFunctionType

    nc = bacc.Bacc("TRN2", target_bir_lowering=False, debug=False,
                   num_devices=NCORES)

    X_d = nc.dram_tensor("X", [SHARD, C], f32, kind="ExternalInput")
    d_d = nc.dram_tensor("d", [SHARD], i32, kind="ExternalInput")
    g_d = nc.dram_tensor("gamma", [D, C], f32, kind="ExternalInput")
    b_d = nc.dram_tensor("beta", [D, C], f32, kind="ExternalInput")
    Y_d = nc.dram_tensor("Y", [SHARD, C], f16, kind="ExternalOutput")

    cc_in = nc.dram_tensor("cc_in", [D, 2 * C + 1], f16)
    cc_out = nc.dram_tensor("cc_out", [D, 2 * C + 1], f16, addr_space="Shared")

    # partition p owns rows [p*64, (p+1)*64): per-partition contiguous DMA
    Xv = X_d.ap().rearrange("(p n) c -> p n c", p=P)   # [128, 64, 512]
    Yv = Y_d.ap().rearrange("(p n) c -> p n c", p=P)

    DB = 1024  # d-broadcast strip width

    with tile.TileContext(nc) as tc:
        with (
            tc.tile_pool(name="const", bufs=1) as cpool,
            tc.tile_pool(name="x", bufs=4) as xpool,
            tc.tile_pool(name="xb", bufs=SUPERS) as xbpool,
            tc.tile_pool(name="sq", bufs=4) as sqpool,
            tc.tile_pool(name="oh", bufs=1) as ohpool,
            tc.tile_pool(name="small", bufs=1) as spool,
            tc.tile_pool(name="scr", bufs=2) as scrpool,
            tc.tile_pool(name="dbc", bufs=2) as dbcpool,
            tc.tile_pool(name="pt", bufs=4) as ptpool,
            tc.tile_pool(name="y", bufs=4) as ypool,
        ):
            # ---- constants ----
            # iota_rep[p, i, j] = j  (for the chunk-layout one-hot)
            iota_rep = cpool.tile([P, CHUNKS, D], f16)
            nc.gpsimd.iota(iota_rep[:], pattern=[[0, CHUNKS], [1, D]], base=0,
                           channel_multiplier=0,
                           allow_small_or_imprecise_dtypes=True)
            # iota_col32[p, 0] = p % 16 as f32 (for the transposed one-hot)
            iota_i = cpool.tile([KT, 1], i32)
            nc.gpsimd.iota(iota_i[:], pattern=[[0, 1]], base=0,
                           channel_multiplier=1)
            nc.vector.tensor_scalar(iota_i[:], iota_i[:], D - 1, None,
                                    Alu.bitwise_and)
            iota_col32 = cpool.tile([KT, 1], f32)
            nc.vector.tensor_copy(iota_col32[:], iota_i[:])
            ones_col = cpool.tile([P, 1], f16)
            nc.vector.memset(ones_col[:], 1.0)
            epsb = cpool.tile([D, 1], f32)
            nc.vector.memset(epsb[:], EPS)

            # ---- d in chunk layout ([p, n]) and one-hot [128, 64, 16] ----
            d_pn = cpool.tile([P, CHUNKS], i32)
            nc.sync.dma_start(d_pn[:], d_d.ap().rearrange("(p n) -> p n", p=P))
            d_f = cpool.tile([P, CHUNKS], f16)
            nc.vector.tensor_copy(d_f[:], d_pn[:])
            onehot = ohpool.tile([P, CHUNKS, D], f16)
            nc.vector.tensor_tensor(
                onehot[:], iota_rep[:],
                d_f[:].unsqueeze(-1).broadcast_to([P, CHUNKS, D]),
                Alu.is_equal)

            # ---- transposed one-hot, rows 0:16 real / 16:32 dup (their
            # table rows in A2/B2 are zero); gather matmuls use K=32.
            # Strips are built lazily inside the phase-1 loop (DVE slack),
            # NOT on gpsimd: its elementwise path is ~20x slower and it
            # shares the DVE SBUF port. ----
            onehotT = ohpool.tile([KT, SHARD], f16)

            def emit_strip(h):
                d_bc = dbcpool.tile([KT, DB], i32)
                src = d_d.ap()[h * DB:(h + 1) * DB]
                src = src.rearrange("(a n) -> a n", a=1).partition_broadcast(KT)
                nc.gpsimd.dma_start(d_bc[:], src)
                nc.vector.tensor_scalar(onehotT[:, h * DB:(h + 1) * DB],
                                        d_bc[:], iota_col32[:], None,
                                        Alu.is_equal)

            # fp16 A/B tables, rows 16:32 zero
            A2 = spool.tile([KT, C], f16, tag="A2")
            B2 = spool.tile([KT, C], f16, tag="B2")
            nc.vector.memset(A2[:], 0.0)
            nc.vector.memset(B2[:], 0.0)

            # gamma/beta in early (needed post-AR)
            gam = spool.tile([D, C], f32, tag="gam")
            nc.scalar.dma_start(gam[:], g_d[:])
            bet = spool.tile([D, C], f32, tag="bet")
            nc.scalar.dma_start(bet[:], b_d[:])

            # preload the Square act table while everything else boots
            tdummy = spool.tile([1, 1], f32, tag="tdummy")
            nc.scalar.activation(tdummy[:], epsb[0:1, :], Act.Square)

            # ---- phase 1: per-core partial stats ----
            stats = spool.tile([D, 2 * C + 1], f16, tag="stats")
            xbs = []
            with tc.tile_pool(name="ps1", bufs=1, space="PSUM") as ps1:
                psum_s = ps1.tile([D, C], f32)
                psum_q = ps1.tile([D, C], f32)
                psum_c = ps1.tile([D, 1], f32)
                warm = ps1.tile([D, C], f32)

                # wake the PE HAM clock-gate with a back-to-back junk burst
                # (~5us busy) so the DMA-paced stats matmuls run at 2.4 GHz
                jm = spool.tile([P, C], f16, tag="jm")
                nc.vector.memset(jm[:], 1.0)
                for _ in range(8):
                    nc.tensor.matmul(warm[:], iota_rep[:, 0, :], jm[:],
                                     start=True, stop=True,
                                     skip_group_check=True)

                # counts early: reduce one-hot over chunks, then one matmul
                rowcnt = spool.tile([P, D], f32, tag="rowcnt")
                nc.vector.tensor_reduce(
                    rowcnt[:], onehot[:].rearrange("p n d -> p d n"),
                    mybir.AxisListType.X, Alu.add)
                rowcnt16 = spool.tile([P, D], f16, tag="rowcnt16")
                nc.vector.tensor_copy(rowcnt16[:], rowcnt[:])
                nc.tensor.matmul(psum_c[:], rowcnt16[:], ones_col[:],
                                 start=True, stop=True)
                nc.scalar.activation(stats[:, 2 * C:2 * C + 1], psum_c[:],
                                     Act.Copy)

                strip_at = {2 + 3 * h: h for h in range(SHARD // DB)}
                for s in range(SUPERS):
                    xt = xpool.tile([P, 2 * C], f32)
                    nc.sync.dma_start(
                        xt[:].rearrange("p (n c) -> p n c", c=C),
                        Xv[:, 2 * s:2 * s + 2, :])
                    xb = xbpool.tile([P, 2 * C], f16)
                    xbs.append(xb)
                    nc.vector.tensor_copy(xb[:], xt[:])
                    xq = sqpool.tile([P, 2 * C], f16, tag="xq")
                    if s == 0 or s % 2 == 1:
                        nc.vector.tensor_mul(xq[:], xb[:], xb[:])
                    else:
                        nc.scalar.activation(xq[:], xt[:], Act.Square)
                    if s in strip_at:
                        emit_strip(strip_at[s])
                    for k in range(2):
                        i = 2 * s + k
                        oh = onehot[:, i, :]
                        st, sp = (i == 0), (i == CHUNKS - 1)
                        csl = slice(k * C, (k + 1) * C)
                        nc.tensor.matmul(psum_s[:], oh, xb[:, csl],
                                         start=st, stop=sp)
                        nc.tensor.matmul(psum_q[:], oh, xq[:, csl],
                                         start=st, stop=sp)

                # ---- pack fp16 stats out of PSUM (ScalarE: DVE still
                # busy with the last super, and ScalarE converts at 1x) ----
                nc.scalar.activation(stats[:, 0:C], psum_s[:], Act.Copy)
                nc.scalar.activation(stats[:, C:2 * C], psum_q[:], Act.Copy)

                # ---- all-reduce partial stats across the 8 cores ----
                nc.sync.dma_start(cc_in[:], stats[:])
                nc.gpsimd.collective_compute(
                    "AllReduce", Alu.add,
                    replica_groups=[list(range(NCORES))],
                    ins=[cc_in[:]], outs=[cc_out[:]])

                # keep the PE HAM clock-gate warm across the all-reduce
                # stall: junk matmuls chained through ScalarE copies so
                # they spread over the stall instead of firing at once
                wsb = spool.tile([D, C], f16, tag="wsb")
                nc.vector.memset(wsb[:], 1.0)
                for w in range(24):
                    nc.tensor.matmul(warm[:], onehot[:, w, :],
                                     xbs[0][:, 0:C],
                                     start=True, stop=True,
                                     skip_group_check=True)
                    nc.scalar.activation(wsb[:], warm[:], Act.Copy)

                # ScalarE: preload Log/Exp tables during the AR stall
                tdummy = spool.tile([1, 1], f32, tag="tdummy")
                nc.scalar.activation(tdummy[:], epsb[0:1, :], Act.Ln,
                                     bias=epsb[0:1, :])
                nc.scalar.activation(tdummy[:], tdummy[:], Act.Exp)

            red = spool.tile([D, 2 * C + 1], f16, tag="red")
            nc.sync.dma_start(red[:], cc_out[:])

            # ---- finalize: A = inv*gamma, B = beta - mean*A ----
            redf = spool.tile([D, 2 * C + 1], f32, tag="redf")
            nc.vector.tensor_copy(redf[:], red[:])
            cntc = spool.tile([D, 1], f32, tag="cntc")
            nc.vector.tensor_scalar_max(cntc[:], redf[:, 2 * C:2 * C + 1], 1.0)
            rinv = spool.tile([D, 1], f32, tag="rinv")
            nc.vector.reciprocal(rinv[:], cntc[:])
            mean = spool.tile([D, C], f32, tag="mean")
            nc.vector.tensor_scalar_mul(mean[:], redf[:, 0:C], rinv[:])
            var = spool.tile([D, C], f32, tag="var")
            nc.vector.tensor_scalar_mul(var[:], redf[:, C:2 * C], rinv[:])
            negm2 = scrpool.tile([D, C], f32, tag="scr")
            nc.vector.scalar_tensor_tensor(negm2[:], mean[:], -1.0, mean[:],
                                           Alu.mult, Alu.mult)
            nc.vector.tensor_add(var[:], var[:], negm2[:])
            # inv = exp(-0.5 * log(var + eps))  (tables preloaded above)
            lv = scrpool.tile([D, C], f32, tag="scr")
            nc.scalar.activation(lv[:], var[:], Act.Ln, bias=epsb[:])
            inv = spool.tile([D, C], f32, tag="inv")
            nc.scalar.activation(inv[:], lv[:], Act.Exp, scale=-0.5)

            a_t = spool.tile([D, C], f32, tag="a_t")
            nc.vector.tensor_mul(a_t[:], inv[:], gam[:])
            nc.vector.tensor_copy(A2[0:D, :], a_t[:])
            b_t = spool.tile([D, C], f32, tag="b_t")
            nc.vector.scalar_tensor_tensor(b_t[:], mean[:], -1.0, a_t[:],
                                           Alu.mult, Alu.mult)   # -mean*A
            nc.vector.tensor_add(b_t[:], bet[:], b_t[:])
            nc.vector.tensor_copy(B2[0:D, :], b_t[:])

            # ---- phase 2: gather A/B per row and normalize ----
            ohT = onehotT[:].rearrange("k (p i) -> k i p", i=CHUNKS)
            with tc.tile_pool(name="ps2", bufs=2, space="PSUM") as ps2:
                for s in range(SUPERS):
                    pa = ps2.tile([P, 2 * C], f32)
                    pb = ps2.tile([P, 2 * C], f32)
                    for k in range(2):
                        i = 2 * s + k
                        lt = ohT[:, i, :]
                        csl = slice(k * C, (k + 1) * C)
                        nc.tensor.matmul(pa[:, csl], lt, A2[:],
                                         start=True, stop=True)
                        nc.tensor.matmul(pb[:, csl], lt, B2[:],
                                         start=True, stop=True)
                    yt = ypool.tile([P, 2 * C], f16)
                    if s % 4 == 1:
                        # DVE consumes PSUM f32 directly (1x mode)
                        nc.vector.tensor_mul(yt[:], xbs[s][:], pa[:])
                        nc.vector.tensor_add(yt[:], yt[:], pb[:])
                    else:
                        # ScalarE drains PSUM to fp16; DVE runs 2x fp16
                        pa16 = ptpool.tile([P, 2 * C], f16)
                        nc.scalar.activation(pa16[:], pa[:], Act.Copy)
                        pb16 = ptpool.tile([P, 2 * C], f16)
                        nc.scalar.activation(pb16[:], pb[:], Act.Copy)
                        nc.vector.tensor_mul(yt[:], xbs[s][:], pa16[:])
                        nc.vector.tensor_add(yt[:], yt[:], pb16[:])
                    nc.sync.dma_start(
                        Yv[:, 2 * s:2 * s + 2, :],
                        yt[:].rearrange("p (n c) -> p n c", c=C))

    nc.compile()
    return nc


def _get_program():
    if "nc" not in _CACHE:
        _CACHE["nc"] = _build_program()
    return _CACHE["nc"]


def kernel(X, d, parameter_t, fm_mean, gamma, beta):
    from concourse.bass_utils import run_bass_kernel_spmd

    X = np.ascontiguousarray(np.asarray(X), dtype=np.float32)
    d = np.ascontiguousarray(np.asarray(d), dtype=np.int32)
    gamma = np.ascontiguousarray(np.asarray(gamma), dtype=np.float32)
    beta = np.ascontiguousarray(np.asarray(beta), dtype=np.float32)

    nc = _get_program()
    in_maps = [
        {
            "X": X[c * SHARD:(c + 1) * SHARD],
            "d": d[c * SHARD:(c + 1) * SHARD],
            "gamma": gamma,
            "beta": beta,
        }
        for c in range(NCORES)
    ]
    res = run_bass_kernel_spmd(nc, in_maps, core_ids=list(range(NCORES)))
    out = np.concatenate([res.results[c]["Y"] for c in range(NCORES)], axis=0)
    return out.astype(np.float32, copy=False)


# revision 12
# speedup vs baseline: 1.6127x; 1.0605x over previous
"""Per-domain batch normalization (BaseDomainBatchNorm) on 8 Trainium2 NeuronCores.

Math (reference):
    cnt[j]   = #{n : d[n] == j}            (clamped to >= 1)
    mean[j]  = sum_{d[n]==j} X[n] / cnt[j]
    var[j]   = sum_{d[n]==j} X[n]^2 / cnt[j] - mean[j]^2
    inv[j]   = rsqrt(var[j] + 1e-5)
    Y[n]     = (X[n] - mean[d[n]]) * inv[d[n]] * gamma[d[n]] + beta[d[n]]
             = X[n] * A[d[n]] + B[d[n]],  A = inv*gamma, B = beta - mean*A

Sharding: rows (samples) split 8192 per core; per-domain partial stats
(sum / sumsq / count) are AllReduce'd (fp16 payload) across the 8 cores;
each core then normalizes its own rows.  gamma/beta replicated.

On-core schedule (fp16 data paths, fp32 accumulation):
  phase 1 (DMA-bound): X streams in per 256-row super-chunk; DVE casts
    f32->fp16 (2x two-port mode), squares alternate DVE (fp16 2x) and
    ScalarE (Square activation); stats accumulate in PSUM via one-hot
    fp16 matmuls.  GpSimd builds the transposed one-hot for phase 2.
  all-reduce: [16, 1025] fp16 payload over shared DRAM; PE kept warm by
    a chained junk-matmul ladder; ScalarE preloads Log/Exp act tables.
  finalize: inv = exp(-0.5*log(var+eps)) on ScalarE; A/B tables in fp16.
  phase 2: per chunk one K=32 gather matmul pair (A,B) -> PSUM f32;
    3/4 of supers: ScalarE copies PSUM->fp16, DVE does fp16 2x mul/add;
    1/4 of supers: DVE consumes PSUM f32 directly (1x).  Y leaves as
    fp16 (host upcasts), halving write traffic.
"""

import numpy as np

N = 65536
C = 512
D = 16
NCORES = 8
SHARD = N // NCORES          # 8192 rows per core
P = 128                      # partitions
CHUNKS = SHARD // P          # 64 chunks of 128 rows
SUPERS = CHUNKS // 2         # 32 super-chunks of 256 rows
EPS = 1e-5
KT = 32                      # gather stationary K (table rows 0:16 + zero pad)

_CACHE = {}


def _build_program():
    import concourse.bacc as bacc
    import concourse.bass as bass
    import concourse.tile as tile
    from concourse import mybir

    f32 = mybir.dt.float32
    f16 = mybir.dt.float16
    i32 = mybir.dt.int32
    Alu = mybir.AluOpType
    Act = mybir.ActivationFunctionType

    nc = bacc.Bacc("TRN2", target_bir_lowering=False, debug=False,
                   num_devices=NCORES)

    X_d = nc.dram_tensor("X", [SHARD, C], f32, kind="ExternalInput")
    d_d = nc.dram_tensor("d", [SHARD], i32, kind="ExternalInput")
    g_d = nc.dram_tensor("gamma", [D, C], f32, kind="ExternalInput")
    b_d = nc.dram_tensor("beta", [D, C], f32, kind="ExternalInput")
    Y_d = nc.dram_tensor("Y", [SHARD, C], f16, kind="ExternalOutput")

    cc_in = nc.dram_tensor("cc_in", [D, 2 * C + 1], f16)
    cc_out = nc.dram_tensor("cc_out", [D, 2 * C + 1], f16, addr_space="Shared")

    # partition p owns rows [p*64, (p+1)*64): per-partition contiguous DMA
    Xv = X_d.ap().rearrange("(p n) c -> p n c", p=P)   # [128, 64, 512]
    Yv = Y_d.ap().rearrange("(p n) c -> p n c", p=P)

    DB = 1024  # d-broadcast strip width

    with tile.TileContext(nc) as tc:
        with (
            tc.tile_pool(name="const", bufs=1) as cpool,
            tc.tile_pool(name="x", bufs=4) as xpool,
            tc.tile_pool(name="xb", bufs=SUPERS) as xbpool,
            tc.tile_pool(name="sq", bufs=4) as sqpool,
            tc.tile_pool(name="oh", bufs=1) as ohpool,
            tc.tile_pool(name="small", bufs=1) as spool,
            tc.tile_pool(name="scr", bufs=2) as scrpool,
            tc.tile_pool(name="dbc", bufs=2) as dbcpool,
            tc.tile_pool(name="pt", bufs=4) as ptpool,
            tc.tile_pool(name="y", bufs=4) as ypool,
        ):
            # ---- constants ----
            # iota_rep[p, i, j] = j  (for the chunk-layout one-hot)
            iota_rep = cpool.tile([P, CHUNKS, D], f16)
            nc.gpsimd.iota(iota_rep[:], pattern=[[0, CHUNKS], [1, D]], base=0,
                           channel_multiplier=0,
                           allow_small_or_imprecise_dtypes=True)
            # iota_col32[p, 0] = p % 16 as f32 (for the transposed one-hot)
            iota_i = cpool.tile([KT, 1], i32)
            nc.gpsimd.iota(iota_i[:], pattern=[[0, 1]], base=0,
                           channel_multiplier=1)
            nc.vector.tensor_scalar(iota_i[:], iota_i[:], D - 1, None,
                                    Alu.bitwise_and)
            iota_col32 = cpool.tile([KT, 1], f32)
            nc.vector.tensor_copy(iota_col32[:], iota_i[:])
            ones_col = cpool.tile([P, 1], f16)
            nc.vector.memset(ones_col[:], 1.0)
            epsb = cpool.tile([D, 1], f32)
            nc.vector.memset(epsb[:], EPS)

            # ---- d in chunk layout ([p, n]) and one-hot [128, 64, 16] ----
            d_pn = cpool.tile([P, CHUNKS], i32)
            nc.sync.dma_start(d_pn[:], d_d.ap().rearrange("(p n) -> p n", p=P))
            d_f = cpool.tile([P, CHUNKS], f16)
            nc.vector.tensor_copy(d_f[:], d_pn[:])
            onehot = ohpool.tile([P, CHUNKS, D], f16)
            nc.vector.tensor_tensor(
                onehot[:], iota_rep[:],
                d_f[:].unsqueeze(-1).broadcast_to([P, CHUNKS, D]),
                Alu.is_equal)

            # ---- transposed one-hot, rows 0:16 real / 16:32 dup (their
            # table rows in A2/B2 are zero); gather matmuls use K=32.
            # Strips are built lazily inside the phase-1 loop (DVE slack),
            # NOT on gpsimd: its elementwise path is ~20x slower and it
            # shares the DVE SBUF port. ----
            onehotT = ohpool.tile([KT, SHARD], f16)

            def emit_strip(h):
                d_bc = dbcpool.tile([KT, DB], i32)
                src = d_d.ap()[h * DB:(h + 1) * DB]
                src = src.rearrange("(a n) -> a n", a=1).partition_broadcast(KT)
                nc.gpsimd.dma_start(d_bc[:], src)
                nc.vector.tensor_scalar(onehotT[:, h * DB:(h + 1) * DB],
                                        d_bc[:], iota_col32[:], None,
                                        Alu.is_equal)

            # fp16 A/B tables, rows 16:32 zero
            A2 = spool.tile([KT, C], f16, tag="A2")
            B2 = spool.tile([KT, C], f16, tag="B2")
            nc.vector.memset(A2[:], 0.0)
            nc.vector.memset(B2[:], 0.0)

            # gamma/beta in early (needed post-AR)
            gam = spool.tile([D, C], f32, tag="gam")
            nc.scalar.dma_start(gam[:], g_d[:])
            bet = spool.tile([D, C], f32, tag="bet")
            nc.scalar.dma_start(bet[:], b_d[:])

            # preload the Square act table while everything else boots
            tdummy = spool.tile([1, 1], f32, tag="tdummy")
            nc.scalar.activation(tdummy[:], epsb[0:1, :], Act.Square)

            # ---- phase 1: per-core partial stats ----
            stats = spool.tile([D, 2 * C + 1], f16, tag="stats")
            xbs = []
            with tc.tile_pool(name="ps1", bufs=1, space="PSUM") as ps1:
                psum_s = ps1.tile([D, C], f32)
                psum_q = ps1.tile([D, C], f32)
                psum_c = ps1.tile([D, 1], f32)
                warm = ps1.tile([D, C], f32)

                # wake the PE HAM clock-gate with a back-to-back junk burst
                # (~5us busy) so the DMA-paced stats matmuls run at 2.4 GHz
                jm = spool.tile([P, C], f16, tag="jm")
                nc.vector.memset(jm[:], 1.0)
                for _ in range(8):
                    nc.tensor.matmul(warm[:], jm[:, 0:D], jm[:],
                                     start=True, stop=True,
                                     skip_group_check=True)

                # counts early: reduce one-hot over chunks, then one matmul
                rowcnt = spool.tile([P, D], f32, tag="rowcnt")
                nc.vector.tensor_reduce(
                    rowcnt[:], onehot[:].rearrange("p n d -> p d n"),
                    mybir.AxisListType.X, Alu.add)
                rowcnt16 = spool.tile([P, D], f16, tag="rowcnt16")
                nc.vector.tensor_copy(rowcnt16[:], rowcnt[:])
                nc.tensor.matmul(psum_c[:], rowcnt16[:], ones_col[:],
                                 start=True, stop=True)
                nc.scalar.activation(stats[:, 2 * C:2 * C + 1], psum_c[:],
                                     Act.Copy)

                strip_at = {2 + 3 * h: h for h in range(SHARD // DB)}
                for s in range(SUPERS):
                    xt = xpool.tile([P, 2 * C], f32)
                    nc.sync.dma_start(
                        xt[:].rearrange("p (n c) -> p n c", c=C),
                        Xv[:, 2 * s:2 * s + 2, :])
                    xb = xbpool.tile([P, 2 * C], f16)
                    xbs.append(xb)
                    nc.vector.tensor_copy(xb[:], xt[:])
                    xq = sqpool.tile([P, 2 * C], f16, tag="xq")
                    if s == 0:
                        nc.vector.tensor_mul(xq[:], xb[:], xb[:])
                    else:
                        nc.scalar.activation(xq[:], xt[:], Act.Square)
                    if s in strip_at:
                        emit_strip(strip_at[s])
                    for k in range(2):
                        i = 2 * s + k
                        oh = onehot[:, i, :]
                        st, sp = (i == 0), (i == CHUNKS - 1)
                        csl = slice(k * C, (k + 1) * C)
                        nc.tensor.matmul(psum_s[:], oh, xb[:, csl],
                                         start=st, stop=sp)
                        nc.tensor.matmul(psum_q[:], oh, xq[:, csl],
                                         start=st, stop=sp)

                # ---- pack fp16 stats out of PSUM (ScalarE: DVE still
                # busy with the last super, and ScalarE converts at 1x) ----
                nc.scalar.activation(stats[:, 0:C], psum_s[:], Act.Copy)
                nc.scalar.activation(stats[:, C:2 * C], psum_q[:], Act.Copy)

                # ---- all-reduce partial stats across the 8 cores ----
                nc.sync.dma_start(cc_in[:], stats[:])
                nc.gpsimd.collective_compute(
                    "AllReduce", Alu.add,
                    replica_groups=[list(range(NCORES))],
                    ins=[cc_in[:]], outs=[cc_out[:]])

                # keep the PE HAM clock-gate warm across the all-reduce
                # stall: junk matmuls chained through ScalarE copies so
                # they spread over the stall instead of firing at once
                wsb = spool.tile([D, C], f16, tag="wsb")
                nc.vector.memset(wsb[:], 1.0)
                for w in range(24):
                    nc.tensor.matmul(warm[:], onehot[:, w, :],
                                     xbs[0][:, 0:C],
                                     start=True, stop=True,
                                     skip_group_check=True)
                    nc.scalar.activation(wsb[:], warm[:], Act.Copy)

                # ScalarE: preload the rsqrt act table during the AR stall
                nc.scalar.activation(tdummy[:], epsb[0:1, :],
                                     Act.Abs_reciprocal_sqrt,
                                     bias=epsb[0:1, :])

            red = spool.tile([D, 2 * C + 1], f16, tag="red")
            nc.sync.dma_start(red[:], cc_out[:])

            # ---- finalize: A = gamma*rsqrt(var+eps), B = beta - mean*A
            # (Abs_reciprocal_sqrt table preloaded during the AR) ----
            cntc = spool.tile([D, 1], f32, tag="cntc")
            nc.vector.tensor_scalar_max(cntc[:], red[:, 2 * C:2 * C + 1], 1.0)
            rinv = spool.tile([D, 1], f32, tag="rinv")
            nc.vector.reciprocal(rinv[:], cntc[:])
            mean = spool.tile([D, C], f32, tag="mean")
            nc.vector.tensor_scalar_mul(mean[:], red[:, 0:C], rinv[:])
            var = spool.tile([D, C], f32, tag="var")
            nc.vector.tensor_scalar_mul(var[:], red[:, C:2 * C], rinv[:])
            negm2 = scrpool.tile([D, C], f32, tag="scr")
            nc.vector.scalar_tensor_tensor(negm2[:], mean[:], -1.0, mean[:],
                                           Alu.mult, Alu.mult)
            nc.vector.tensor_add(var[:], var[:], negm2[:])
            dsq = spool.tile([D, C], f32, tag="dsq")
            nc.scalar.activation(dsq[:], var[:], Act.Abs_reciprocal_sqrt,
                                 bias=epsb[:])
            a_t = spool.tile([D, C], f32, tag="a_t")
            nc.vector.tensor_mul(a_t[:], dsq[:], gam[:])
            nc.scalar.activation(A2[0:D, :], a_t[:], Act.Copy)
            b_t = spool.tile([D, C], f32, tag="b_t")
            nc.vector.scalar_tensor_tensor(b_t[:], mean[:], -1.0, a_t[:],
                                           Alu.mult, Alu.mult)   # -mean*A
            nc.vector.tensor_add(b_t[:], bet[:], b_t[:])
            nc.scalar.activation(B2[0:D, :], b_t[:], Act.Copy)

            # ---- phase 2: gather A/B per row and normalize ----
            ohT = onehotT[:].rearrange("k (p i) -> k i p", i=CHUNKS)
            with tc.tile_pool(name="ps2", bufs=2, space="PSUM") as ps2:
                for s in range(SUPERS):
                    pa = ps2.tile([P, 2 * C], f32)
                    pb = ps2.tile([P, 2 * C], f32)
                    for k in range(2):
                        i = 2 * s + k
                        lt = ohT[:, i, :]
                        csl = slice(k * C, (k + 1) * C)
                        nc.tensor.matmul(pa[:, csl], lt, A2[:],
                                         start=True, stop=True)
                        nc.tensor.matmul(pb[:, csl], lt, B2[:],
                                         start=True, stop=True)
                    yt = ypool.tile([P, 2 * C], f16)
                    if s % 4 == 1:
                        # DVE consumes PSUM f32 directly (1x mode)
                        nc.vector.tensor_mul(yt[:], xbs[s][:], pa[:])
                        nc.vector.tensor_add(yt[:], yt[:], pb[:])
                    else:
                        # ScalarE drains PSUM to fp16; DVE runs 2x fp16
                        pa16 = ptpool.tile([P, 2 * C], f16)
                        nc.scalar.activation(pa16[:], pa[:], Act.Copy)
                        pb16 = ptpool.tile([P, 2 * C], f16)
                        nc.scalar.activation(pb16[:], pb[:], Act.Copy)
                        nc.vector.tensor_mul(yt[:], xbs[s][:], pa16[:])
                        nc.vector.tensor_add(yt[:], yt[:], pb16[:])
                    nc.sync.dma_start(
                        Yv[:, 2 * s:2 * s + 2, :],
                        yt[:].rearrange("p (n c) -> p n c", c=C))

    nc.compile()
    return nc


def _get_program():
    if "nc" not in _CACHE:
        _CACHE["nc"] = _build_program()
    return _CACHE["nc"]


def kernel(X, d, parameter_t, fm_mean, gamma, beta):
    from concourse.bass_utils import run_bass_kernel_spmd

    X = np.ascontiguousarray(np.asarray(X), dtype=np.float32)
    d = np.ascontiguousarray(np.asarray(d), dtype=np.int32)
    gamma = np.ascontiguousarray(np.asarray(gamma), dtype=np.float32)
    beta = np.ascontiguousarray(np.asarray(beta), dtype=np.float32)

    nc = _get_program()
    in_maps = [
        {
            "X": X[c * SHARD:(c + 1) * SHARD],
            "d": d[c * SHARD:(c + 1) * SHARD],
            "gamma": gamma,
            "beta": beta,
        }
        for c in range(NCORES)
    ]
    res = run_bass_kernel_spmd(nc, in_maps, core_ids=list(range(NCORES)))
    out = np.concatenate([res.results[c]["Y"] for c in range(NCORES)], axis=0)
    return out.astype(np.float32, copy=False)
